# revision 40
# baseline (speedup 1.0000x reference)
"""BaiLing attention Trainium2 kernel.

Sharding: 8 cores = 2 (batch) x 4 (tensor-parallel over heads).
Each TP rank r owns q heads 4r..4r+3 and kv head r (GQA group-aligned),
computes its out-projection partial; host sums the 4 partials per batch.

On-chip layout is feature-major [d, s] everywhere:
  - QKV proj:  psum[qkv_col, s] = w_qkv_slice.T @ hidden.T
  - scoresT:   psum[sk, sq]     = k_tile.T @ q_tile    (both [d, *])
  - causal mask: -1e9 accumulated into the scores psum via an
    identity-matmul of a constant mask tile
  - softmax over sk (partition dim): exp on ACT; the denominator is
    built by pair/tree-summing the exp tiles on DVE (bf16, depth<=4
    roundings) and contracting once per (tile,head) with a full-width
    [128,128] ones-matmul into a broadcast [128,512] psum (every
    partition = den).  Small-M one-hot matmuls (the old scheme) run
    ~1.5x slower on PE and stall the following weight load; full-M
    ones matmuls run at peak.  1/den comes from the DVE
    reciprocal_approx_fast custom op (18 significant bits) and the
    normalize is fused into the PSUM->SBUF copy of the PV output.
    (Elementwise denominator accumulation on the GpSimd/Pool engine
    was tried and is ~4x slower than PE/DVE here, plus its SBUF
    traffic slows concurrent PE matmuls.)
  - per-head RMSNorm: sum of squares via the same ones-matmul
    broadcast trick, Ln/Exp on ACT for rsqrt, multiply on DVE.
  - PV:        psum[d, sq]      = vT_tile.T @ probsT
  - out-proj:  psum[s, n]       = oT_slice.T @ w_o_slice; partials are
    stored as fp16 (halves the output DMA traffic; host sums in f64).
Out-proj tiles are emitted one attention tile late so the tensor
engine queue never waits on the normalize chain.  Inputs are staged
host-side in SBUF-matching layouts (hiddens st-major, weights
ko-major) so every DMA row is 8-16KB contiguous -- small-row DMAs run
the rings at <40% of peak and starved the first 40us.  DMA issue is
spread across the three DMA-capable rings (sync/scalar/gpsimd),
ordered just-in-time for the compute sequence.
"""

import sys

sys.path.insert(0, "/opt/trn_rl_repo")

import math
from contextlib import ExitStack

import ml_dtypes
import numpy as np

BF = ml_dtypes.bfloat16

import concourse.bass as bass
import concourse.mybir as mybir
import concourse.tile as tile
from concourse import bacc
from concourse.bass_utils import run_bass_kernel_spmd

F32 = mybir.dt.float32
BF16 = mybir.dt.bfloat16
FP16 = mybir.dt.float16
I32 = mybir.dt.int32
AF = mybir.ActivationFunctionType
OP = mybir.AluOpType

H = 2048          # hidden size
S = 2048          # sequence length
D = 128           # head dim
NH_L = 4          # q heads per rank
QW = NH_L * D     # 512 local q width
CW = QW + 2 * D   # 768 local qkv width
P = 128
KO = H // P       # 16 contraction tiles
ST = S // 512     # 4 seq tiles of 512
SM_SCALE = float(D) ** -0.5
EPS = 1e-6
ROPE_THETA = 10000.0
NEG = -1.0e9


def _build():
    nc = bacc.Bacc("TRN2", target_bir_lowering=False, debug=False, num_devices=8)

    # hiddens st-major and weights ko-major so every DMA row is 8-16KB
    # contiguous (512B-1KB rows run the DMA rings at <40% of peak)
    hT = nc.dram_tensor("hT", [ST * P, KO, 512], BF16,
                        kind="ExternalInput").ap()
    wkv = nc.dram_tensor("wkv", [P, KO, 2 * D], BF16,
                         kind="ExternalInput").ap()
    wqz = nc.dram_tensor("wqz", [P, KO, QW], BF16,
                         kind="ExternalInput").ap()
    wo = nc.dram_tensor("wo", [QW, H], BF16, kind="ExternalInput").ap()
    cosbt = nc.dram_tensor("cosbt", [P, S], BF16, kind="ExternalInput").ap()
    sinbt = nc.dram_tensor("sinbt", [P, S], BF16, kind="ExternalInput").ap()
    wqn = nc.dram_tensor("wqn", [D, 1], F32, kind="ExternalInput").ap()
    wkn = nc.dram_tensor("wkn", [D, 1], F32, kind="ExternalInput").ap()
    maskneg = nc.dram_tensor("maskneg", [P, P], BF16, kind="ExternalInput").ap()
    identr = nc.dram_tensor("identr", [P, P], BF16, kind="ExternalInput").ap()
    rmat = nc.dram_tensor("rmat", [P, P], BF16, kind="ExternalInput").ap()
    out = nc.dram_tensor("out", [4 * ST * P, H], FP16,
                         kind="ExternalOutput").ap()

    hT4 = hT.rearrange("(st p) ko s -> p st ko s", p=P)
    wo3 = wo.rearrange("(ks p) n -> p ks n", p=P)
    out4 = out.rearrange("(t p) n -> p t n", p=P)

    with ExitStack() as ctx:
        tc = ctx.enter_context(tile.TileContext(nc))
        consts = ctx.enter_context(tc.tile_pool(name="consts", bufs=1))
        qkvp = ctx.enter_context(tc.tile_pool(name="qkvp", bufs=1))
        vtp = ctx.enter_context(tc.tile_pool(name="vtp", bufs=1))
        abp = ExitStack()
        csp = abp.enter_context(tc.tile_pool(name="csp", bufs=1))

        # const tiles are allocated here but their (small, low-priority)
        # DMAs are issued inside the st0 block AFTER the critical
        # hidden/weight stream so they don't delay the first matmuls.
        # cos/sin rope tables are host-computed (exact) and DMA'd in
        # per-st chunks in need-order.
        wqn_sb = consts.tile([D, 1], F32)
        wkn_sb = consts.tile([D, 1], F32)
        rmat_sb = consts.tile([P, P], BF16)
        identr_sb = consts.tile([P, P], BF16)
        maskneg_sb = consts.tile([P, P], BF16)
        cosb = csp.tile([P, S], BF16)
        sinb = csp.tile([P, S], BF16)
        ones_sb = consts.tile([P, P], BF16)
        nc.vector.memset(ones_sb, 1.0)
        eps_sb = consts.tile([P, 1], F32)
        nc.vector.memset(eps_sb, EPS)

        # fine-grained tiles (per head / per seq-tile) so readers only
        # wait on the exact producer, not the whole buffer's last writer
        q_sb = [[qkvp.tile([P, 512], BF16, name=f"q_{h}_{s}")
                 for s in range(ST)] for h in range(NH_L)]
        k_sb = [qkvp.tile([P, 512], BF16, name=f"k_{s}") for s in range(ST)]
        vT_sb = [vtp.tile([P, 4, P], BF16, name=f"vT_{s}")
                 for s in range(ST)]

        wqkv_p = abp.enter_context(tc.tile_pool(name="wqkv_p", bufs=1))
        ht_p = abp.enter_context(tc.tile_pool(name="ht_p", bufs=2))
        cpool = abp.enter_context(tc.tile_pool(name="cpool", bufs=1))

        # ---- Phase A: QKV projection + fused norm/rope/v-transpose ----
        # host layout: wqkv cols = [k(128), v(128), q(512)]
        CT_ORDER = [0, 1, 2, 3, 4, 5]  # k, v, then q heads

        def chunk_stage1(x_ch, w_sb, st, ps_c, ps_q, d2):
            """sum-of-squares via full-width ones-matmul into a broadcast
            [128,512] psum (every partition = ssq), rsqrt via Ln/Exp on
            ACT, then the rope rotation.  Emitted one QKV group late so
            the matmuls' DVE inputs are ready when PE reaches them."""
            sl = slice(512 * st, 512 * (st + 1))
            xsq = cpool.tile([P, 512], BF16, tag="ctmp", bufs=4, name="xsq")
            nc.vector.tensor_mul(xsq, x_ch, x_ch)
            sq_ps = ps_q.tile([P, 512], F32, tag="sq", name="sq_ps")
            nc.tensor.matmul(sq_ps, ones_sb, xsq, start=True, stop=True)
            ln_t = cpool.tile([P, 512], F32, tag="lnt", bufs=2, name="ln_t")
            nc.scalar.activation(ln_t, sq_ps, AF.Ln, bias=eps_sb,
                                 scale=1.0 / D)
            rb = cpool.tile([P, 512], BF16, tag="rb", bufs=3, name="rb")
            nc.scalar.activation(rb, ln_t, AF.Exp, scale=-0.5)
            nc.vector.tensor_scalar_mul(x_ch, x_ch, w_sb)
            t1m = cpool.tile([P, 512], BF16, tag="ctmp", bufs=4, name="t1m")
            nc.vector.tensor_mul(t1m, x_ch, cosb[:, sl])
            qr_ps = ps_c.tile([P, 512], F32, tag="qr", name="qr_ps")
            nc.tensor.matmul(qr_ps, rmat_sb, x_ch, start=True, stop=True)
            nc.vector.tensor_tensor(x_ch, qr_ps, sinb[:, sl], OP.mult)
            nc.vector.tensor_tensor(x_ch, x_ch, t1m, OP.add)

            def s2(x_ch=x_ch, rb=rb):
                nc.vector.tensor_tensor(x_ch, x_ch, rb, OP.mult)
            d2.append(s2)

        with nc.named_scope("qkv_proj"):
            with tc.tile_pool(name="ps_a", bufs=4, space="PSUM") as ps_a, \
                 tc.tile_pool(name="ps_c", bufs=2, space="PSUM") as ps_c, \
                 tc.tile_pool(name="ps_q", bufs=2, space="PSUM") as ps_q:
                wkv_sb = wqkv_p.tile([P, KO, 2 * D], BF16)
                wqz_sb = wqkv_p.tile([P, KO, QW], BF16)
                d1 = []  # stage-1 closures, one group late
                d2 = []  # stage-2 closures, drained one per group
                hts = [ht_p.tile([P, KO, 512], BF16, tag="ht",
                                 name=f"ht_{st}") for st in range(ST)]
                for st in range(ST):
                    ssl = slice(512 * st, 512 * (st + 1))
                    ht_sb = hts[st]
                    if st == 0:
                        # st0's critical stream is split across three DMA
                        # rings (~180GB/s each) pulling from HBM in
                        # parallel, ordered just-in-time for the
                        # k,v,q0..q3 psum group sequence; st1's hiddens
                        # are prefetched on whichever rings free up first.
                        # 2-ko first chunks (2KB rows) for a fast start,
                        # then 4-ko (4KB rows) for bandwidth: finer splits
                        # drop to 1KB rows and halve ring bandwidth
                        nc.sync.dma_start(wkv_sb[:, 0:8], wkv[:, 0:8])
                        nc.scalar.dma_start(ht_sb[:, 0:2], hT4[:, st, 0:2])
                        nc.gpsimd.dma_start(ht_sb[:, 2:6], hT4[:, st, 2:6])
                        nc.sync.dma_start(wkv_sb[:, 8:KO], wkv[:, 8:KO])
                        nc.scalar.dma_start(ht_sb[:, 6:10], hT4[:, st, 6:10])
                        nc.gpsimd.dma_start(ht_sb[:, 10:13], hT4[:, st, 10:13])
                        nc.scalar.dma_start(ht_sb[:, 13:16], hT4[:, st, 13:16])
                        nc.sync.dma_start(wqz_sb[:, 0:4], wqz[:, 0:4])
                        nc.gpsimd.dma_start(wqz_sb[:, 4:8], wqz[:, 4:8])
                        nc.sync.dma_start(wqz_sb[:, 8:12], wqz[:, 8:12])
                        nc.gpsimd.dma_start(wqz_sb[:, 12:KO], wqz[:, 12:KO])
                        # rope tables + small consts trail st0's hiddens on
                        # the scalar ring
                        nc.scalar.dma_start(wqn_sb, wqn)
                        nc.scalar.dma_start(wkn_sb, wkn)
                        nc.scalar.dma_start(rmat_sb, rmat)
                        nc.scalar.dma_start(cosb[:, 0:512], cosbt[:, 0:512])
                        nc.scalar.dma_start(sinb[:, 0:512], sinbt[:, 0:512])
                        nc.scalar.dma_start(identr_sb, identr)
                        nc.scalar.dma_start(maskneg_sb, maskneg)
                        # st1 hidden prefetch rides the two rings that
                        # drain first
                        nc.sync.dma_start(hts[1][:, 0:8], hT4[:, 1, 0:8])
                        nc.scalar.dma_start(hts[1][:, 8:KO], hT4[:, 1, 8:KO])
                        nc.gpsimd.dma_start(cosb[:, 512:1024],
                                            cosbt[:, 512:1024])
                        nc.gpsimd.dma_start(sinb[:, 512:1024],
                                            sinbt[:, 512:1024])
                    elif st >= 2:
                        # later hiddens + their rope tables ride the gpsimd
                        # queue in need-order
                        nc.gpsimd.dma_start(ht_sb, hT4[:, st])
                        nc.gpsimd.dma_start(cosb[:, ssl], cosbt[:, ssl])
                        nc.gpsimd.dma_start(sinb[:, ssl], sinbt[:, ssl])
                    for ct in CT_ORDER:
                        acc = ps_a.tile([P, 512], F32, tag="qkv_ps",
                                        name=f"qkv_ps_{st}_{ct}")
                        for ko in range(KO):
                            if ct < 2:
                                wsl = wkv_sb[:, ko, P * ct : P * (ct + 1)]
                            else:
                                wsl = wqz_sb[:, ko,
                                             P * (ct - 2) : P * (ct - 1)]
                            nc.tensor.matmul(
                                acc,
                                wsl,
                                ht_sb[:, ko],
                                start=(ko == 0),
                                stop=(ko == KO - 1),
                            )
                        # psum->sbuf copies run on ACT (idle here), keeping
                        # DVE for the rope/rms elementwise chain
                        if ct == 0:
                            x_ch, w_sb = k_sb[st], wkn_sb
                            nc.scalar.copy(x_ch, acc)
                        elif ct == 1:
                            vch = cpool.tile([P, 512], BF16, tag="vch", bufs=2,
                                             name="vch")
                            nc.scalar.copy(vch, acc)
                        else:
                            x_ch, w_sb = q_sb[ct - 2][st], wqn_sb
                            nc.scalar.copy(x_ch, acc)

                        # run pipelined stages of earlier chunks
                        if d1:
                            d1.pop(0)()
                        if d2:
                            d2.pop(0)()

                        if ct == 1:
                            def vtrans(vch=vch, st=st):
                                for i in range(4):
                                    vt_ps = ps_c.tile([P, P], BF16, tag="qr",
                                                      name="vt_ps")
                                    nc.tensor.transpose(
                                        vt_ps, vch[:, P * i : P * (i + 1)],
                                        identr_sb)
                                    nc.vector.tensor_copy(
                                        vT_sb[st][:, i], vt_ps)
                            d1.append(vtrans)
                        else:
                            def s1(x_ch=x_ch, w_sb=w_sb, st=st):
                                chunk_stage1(x_ch, w_sb, st, ps_c, ps_q, d2)
                            d1.append(s1)
                while d1:
                    d1.pop(0)()
                while d2:
                    d2.pop(0)()

        abp.close()  # release cos/sin tables + norm temps

        # ------------- Phase E/F: attention + out projection -------------
        with tc.tile_pool(name="otp", bufs=1) as otp, \
             tc.tile_pool(name="wop", bufs=1) as wop, \
             tc.tile_pool(name="expp", bufs=12) as expp, \
             tc.tile_pool(name="prp", bufs=16) as prp, \
             tc.tile_pool(name="dip", bufs=2) as dip, \
             tc.tile_pool(name="outp", bufs=3) as outp, \
             tc.tile_pool(name="ps_st", bufs=2, space="PSUM") as ps_st, \
             tc.tile_pool(name="ps_o", bufs=2, space="PSUM") as ps_o, \
             tc.tile_pool(name="ps_dn", bufs=2, space="PSUM") as ps_dn:
            oT_sb = [otp.tile([P, S], BF16, name=f"oT_{h}")
                     for h in range(NH_L)]
            wo_sb = wop.tile([P, NH_L, H], BF16)
            for ks in range(NH_L):
                nc.sync.dma_start(wo_sb[:, ks], wo3[:, ks])

            carry = []  # deferred pv/den/out-proj work from previous heads

            def attn_tile(st, hh):
                n_sk = 4 * st + 4
                qh = q_sb[hh][st]
                o_ps = ps_o.tile([P, 512], F32, tag="o_ps", name="o_ps")
                den_ps = ps_dn.tile([P, 512], F32, tag="dn", name="den_ps")

                def vis0(j):
                    # first visible sq column of sk-block j within this
                    # 512-wide sq tile; columns below it are fully masked
                    # and never computed/read anywhere
                    return max(0, 128 * (j - 4 * st))

                def emit_den(eps):
                    # st0-only: denominator via full-width ones-matmul
                    # broadcast of each exp tile's partition sum into every
                    # psum partition (full PE rate, unlike small-M
                    # one-hots).  For st>0 the exp tiles are tree-summed on
                    # DVE instead and contracted with a single matmul per
                    # head (see below).
                    for ep, j0, _pr in eps:
                        for u in (0, 1):
                            j = j0 + u
                            c0 = vis0(j)
                            nc.tensor.matmul(den_ps[:, c0:512], ones_sb,
                                             ep[:, 512 * u + c0 :
                                                 512 * u + 512],
                                             start=(j == 0),
                                             stop=(j == n_sk - 1))

                def emit_pv(eps):
                    for ep, j0, _pr in eps:
                        for u in (0, 1):
                            j = j0 + u
                            c0 = vis0(j)
                            nc.tensor.matmul(o_ps[:, c0:512],
                                             vT_sb[j // 4][:, j % 4],
                                             ep[:, 512 * u + c0 :
                                                 512 * u + 512],
                                             start=(j == 0),
                                             stop=(j == n_sk - 1))

                pend = []
                prs = []
                epl = []
                quads = []
                for m in range(2 * st + 2):
                    sT = ps_st.tile([P, 1024], F32, tag="sT", name="sT")
                    for u in (0, 1):
                        j = 2 * m + u
                        half = sT[:, 512 * u : 512 * (u + 1)]
                        if j >= 4 * st:
                            # scores over the visible span (start marks the
                            # whole bank), then the [128,128] staircase mask
                            # accumulated on the diagonal block only
                            c0 = vis0(j)
                            nc.tensor.matmul(
                                half[:, c0:512],
                                k_sb[j // 4][:, P * (j % 4) : P * (j % 4 + 1)],
                                qh[:, c0:512],
                                start=True, stop=False)
                            nc.tensor.matmul(
                                half[:, c0 : c0 + 128], identr_sb,
                                maskneg_sb,
                                start=False, stop=True)
                        else:
                            nc.tensor.matmul(
                                half,
                                k_sb[j // 4][:, P * (j % 4) : P * (j % 4 + 1)],
                                qh,
                                start=True, stop=True)
                    ep = expp.tile([P, 1024], BF16, tag="ep", name="ep")
                    # the final staircase tile's first 256 columns are
                    # never read; skip their exp
                    lo = 256 if m == 2 * st + 1 else 0
                    nc.scalar.activation(ep[:, lo:1024], sT[:, lo:1024],
                                         AF.Exp, scale=SM_SCALE)
                    # DVE pre-sum of the tile's two halves for the
                    # denominator (skipped at st0 where every block is on
                    # the staircase and the raw path is already cheap)
                    if st > 0:
                        c1 = vis0(2 * m + 1)
                        pr = prp.tile([P, 512], BF16, tag="pr", name="pr")
                        nc.vector.tensor_tensor(pr[:, c1:512],
                                                ep[:, c1:512],
                                                ep[:, 512 + c1 : 1024],
                                                OP.add)
                    else:
                        pr = None
                    # interleave the previous head's deferred den/PV (its
                    # exps are long done) with this head's scores, so PE
                    # never drains a tile's den/PV right after its exps
                    pend.append((ep, 2 * m, pr))
                    prs.append(pr)
                    epl.append(ep)
                    # fold pair m-1 + pair m early (spreads the DVE tree
                    # work across the m loop instead of bursting at the
                    # head boundary)
                    if st > 0 and m % 2 == 1 and m < 2 * st:
                        qd = prp.tile([P, 512], BF16, tag="pr", name="qd")
                        nc.vector.tensor_tensor(qd, prs[m - 1], prs[m],
                                                OP.add)
                        quads.append(qd)
                    if carry:
                        carry.pop(0)()
                    elif len(pend) > 3:
                        batch, pend = pend[:2], pend[2:]
                        if st == 0:
                            emit_den(batch)
                        emit_pv(batch)

                def mk(batch):
                    def go():
                        if st == 0:
                            emit_den(batch)
                        emit_pv(batch)
                    return go

                while pend:
                    batch, pend = pend[:2], pend[2:]
                    carry.append(mk(batch))

                osl = oT_sb[hh][:, 512 * st : 512 * (st + 1)]

                dsum = None
                if st > 0:
                    # tree-sum the pair tiles on DVE (depth <= 4 bf16
                    # roundings) so the head's whole denominator costs a
                    # single full-width PE matmul
                    lvl = quads if quads else prs[: 2 * st]
                    while len(lvl) > 1:
                        nxt = []
                        for i in range(0, len(lvl) - 1, 2):
                            tr = prp.tile([P, 512], BF16, tag="pr",
                                          name="tr")
                            nc.vector.tensor_tensor(tr, lvl[i], lvl[i + 1],
                                                    OP.add)
                            nxt.append(tr)
                        if len(lvl) % 2:
                            nxt.append(lvl[-1])
                        lvl = nxt
                    dsum = lvl[0]
                    # fold in the two staircase tiles' visible regions
                    nc.vector.tensor_tensor(
                        dsum[:, 128:512], dsum[:, 128:512],
                        prs[2 * st][:, 128:512], OP.add)
                    nc.vector.tensor_tensor(
                        dsum[:, 0:128], dsum[:, 0:128],
                        epl[2 * st][:, 0:128], OP.add)
                    nc.vector.tensor_tensor(
                        dsum[:, 384:512], dsum[:, 384:512],
                        prs[2 * st + 1][:, 384:512], OP.add)
                    nc.vector.tensor_tensor(
                        dsum[:, 256:384], dsum[:, 256:384],
                        epl[2 * st + 1][:, 256:384], OP.add)

                def fin_head(den_ps=den_ps, o_ps=o_ps, osl=osl, dsum=dsum):
                    # 1/den on DVE (18-bit custom op), normalize fused into
                    # the PV psum->sbuf copy
                    if dsum is not None:
                        nc.tensor.matmul(den_ps, ones_sb, dsum,
                                         start=True, stop=True)
                    di = dip.tile([P, 512], F32, tag="di", name="di")
                    nc.vector.reciprocal_approx_fast(di, den_ps)
                    nc.vector.tensor_tensor(osl, o_ps, di, OP.mult)
                carry.append(fin_head)
                return osl

            def out_proj(st):
                # the final tile's out-proj is the serial tail: attention is
                # done, so spread its psum groups over the idle scores slots
                # and its copies over both ACT and DVE; the tail's stores go
                # out in halves on alternating queues so the last DMAs don't
                # all trail the final copy
                tail = st == ST - 1
                with nc.named_scope(f"out_proj_t{st}"):
                    for ti in range(4):
                        t = 4 * st + ti
                        ob = outp.tile([P, H], FP16, tag="out_sb",
                                       name="out_sb")
                        for nt in range(4):
                            if tail and nt % 2 == 1:
                                acc = ps_st.tile([P, 512], F32, tag="sT",
                                                 name="out_ps_b")
                            else:
                                acc = ps_o.tile([P, 512], F32, tag="o_ps",
                                                name="out_ps")
                            for ks in range(NH_L):
                                nc.tensor.matmul(
                                    acc,
                                    oT_sb[ks][:, P * t : P * (t + 1)],
                                    wo_sb[:, ks, 512 * nt : 512 * (nt + 1)],
                                    start=(ks == 0),
                                    stop=(ks == NH_L - 1),
                                )
                            dst = ob[:, 512 * nt : 512 * (nt + 1)]
                            # all out copies on ACT: DVE casts reading psum
                            # were stalling concurrent matmul psum writes
                            # (~380ns vs 216ns), and ACT has phase-B slack
                            nc.scalar.copy(dst, acc)
                            if tail and nt % 2 == 1:
                                q = nc.sync if nt == 1 else nc.scalar
                                q.dma_start(
                                    out4[:, t, 512 * (nt - 1) : 512 * (nt + 1)],
                                    ob[:, 512 * (nt - 1) : 512 * (nt + 1)])
                        if not tail:
                            nc.sync.dma_start(out4[:, t], ob)

            for st in range(ST):
                for hh in range(NH_L):
                    with nc.named_scope(f"attn_h{hh}_t{st}"):
                        attn_tile(st, hh)

                def fin_st(st=st):
                    out_proj(st)
                carry.append(fin_st)
            while carry:
                carry.pop(0)()

    nc.compile()
    _merge_act_table_loads(nc)
    return nc


def _merge_act_table_loads(nc):
    """Ln(5)/Exp+Copy(0) both live in set 6 (natural_log_exp_and_others);
    bass's per-function table choice alternates 5/0 and reloads tables at
    every Ln<->Exp transition (~1.5us each).  Retarget those loads to
    set 6 and drop the now-redundant reloads."""
    for b in nc.main_func.blocks:
        loaded = None
        keep = []
        for inst in b.instructions:
            if isinstance(inst, mybir.InstLoadActFuncSet):
                tid = inst.act_func_set_id
                if tid in (0, 5):
                    tid = 6
                if tid == loaded:
                    continue
                inst.act_func_set_id = tid
                loaded = tid
            keep.append(inst)
        b.instructions[:] = keep


_NC_CACHE = None


def _get_nc():
    global _NC_CACHE
    if _NC_CACHE is None:
        _NC_CACHE = _build()
    return _NC_CACHE


def _host_inputs(positions, hidden_states, w_qkv, w_o, q_norm_w, k_norm_w):
    """Build the 8 per-core input maps."""
    positions = np.asarray(positions, dtype=np.int32)
    hidden_states = np.asarray(hidden_states, dtype=np.float32)
    w_qkv = np.asarray(w_qkv, dtype=np.float32)
    w_o = np.asarray(w_o, dtype=np.float32)
    q_norm_w = np.asarray(q_norm_w, dtype=np.float32)
    k_norm_w = np.asarray(k_norm_w, dtype=np.float32)

    invf = 1.0 / (ROPE_THETA ** (np.arange(0, D, 2, dtype=np.float64) / D))
    p_idx = np.arange(P).reshape(P, 1)
    c_idx = np.arange(P).reshape(1, P)
    maskneg = np.where(p_idx > c_idx, np.float32(NEG), np.float32(0.0))
    maskneg = maskneg.astype(BF)
    identr = np.eye(P, dtype=BF)
    rmat = np.zeros((P, P), dtype=BF)
    for i in range(64):
        rmat[64 + i, i] = -1.0
        rmat[i, 64 + i] = 1.0
    wqn = q_norm_w.reshape(D, 1)
    wkn = k_norm_w.reshape(D, 1)

    # host-exact rope tables per batch: row r (r%64 = freq) x position
    cosbt, sinbt = [], []
    for g in range(positions.shape[0]):
        ang = np.outer(invf, positions[g].astype(np.float64))  # [64, S]
        c = np.cos(ang).astype(BF)
        s = np.sin(ang).astype(BF)
        cosbt.append(np.concatenate([c, c], axis=0))
        sinbt.append(np.concatenate([s, s], axis=0))

    # hiddens st-major [ST*P, KO, 512] so each per-st DMA has 16KB
    # contiguous rows
    hT_st = []
    for g in range(hidden_states.shape[0]):
        a = hidden_states[g].T.reshape(KO, P, ST, 512)
        hT_st.append(
            np.ascontiguousarray(a.transpose(2, 1, 0, 3))
            .reshape(ST * P, KO, 512).astype(BF))

    in_maps = []
    for core in range(8):
        g, r = core // 4, core % 4
        wq_cols = w_qkv[:, 512 * r : 512 * (r + 1)]
        wk_col = w_qkv[:, 2048 + 128 * r : 2048 + 128 * (r + 1)]
        wv_col = w_qkv[:, 2560 + 128 * r : 2560 + 128 * (r + 1)]
        # weights ko-major [P, KO, cols]: 8-16KB contiguous DMA rows
        wkv_h = np.concatenate([wk_col, wv_col], axis=1)  # [H, 256]
        wkv_h = np.ascontiguousarray(
            wkv_h.reshape(KO, P, 2 * D).transpose(1, 0, 2)).astype(BF)
        wqz_h = np.ascontiguousarray(
            wq_cols.reshape(KO, P, QW).transpose(1, 0, 2)).astype(BF)
        in_maps.append(
            {
                "hT": hT_st[g],
                "wkv": wkv_h,
                "wqz": wqz_h,
                "wo": np.ascontiguousarray(
                    w_o[512 * r : 512 * (r + 1), :]
                ).astype(BF),
                "cosbt": cosbt[g],
                "sinbt": sinbt[g],
                "wqn": wqn,
                "wkn": wkn,
                "maskneg": maskneg,
                "identr": identr,
                "rmat": rmat,
            }
        )
    return in_maps


def run(trace=False, **inputs):
    nc = _get_nc()
    in_maps = _host_inputs(**inputs)
    res = run_bass_kernel_spmd(nc, in_maps, core_ids=list(range(8)), trace=trace)
    B = inputs["hidden_states"].shape[0]
    out = np.zeros((B, S, H), dtype=np.float64)
    for core in range(8):
        g = core // 4
        out[g] += res.results[core]["out"].astype(np.float64)
    return out.astype(np.float32), res


def kernel(**inputs):
    out, _ = run(trace=False, **inputs)
    return out


# revision 41
# speedup vs baseline: 1.0035x; 1.0035x over previous
"""BaiLing attention Trainium2 kernel.

Sharding: 8 cores = 2 (batch) x 4 (tensor-parallel over heads).
Each TP rank r owns q heads 4r..4r+3 and kv head r (GQA group-aligned),
computes its out-projection partial; host sums the 4 partials per batch.

On-chip layout is feature-major [d, s] everywhere:
  - QKV proj:  psum[qkv_col, s] = w_qkv_slice.T @ hidden.T
  - scoresT:   psum[sk, sq]     = k_tile.T @ q_tile    (both [d, *])
  - causal mask: -1e9 accumulated into the scores psum via an
    identity-matmul of a constant mask tile
  - softmax over sk (partition dim): exp on ACT; the denominator is
    built by pair/tree-summing the exp tiles on DVE (bf16, depth<=4
    roundings) and contracting once per (tile,head) with a full-width
    [128,128] ones-matmul into a broadcast [128,512] psum (every
    partition = den).  Small-M one-hot matmuls (the old scheme) run
    ~1.5x slower on PE and stall the following weight load; full-M
    ones matmuls run at peak.  1/den comes from the DVE
    reciprocal_approx_fast custom op (18 significant bits) and the
    normalize is fused into the PSUM->SBUF copy of the PV output.
    (Elementwise denominator accumulation on the GpSimd/Pool engine
    was tried and is ~4x slower than PE/DVE here, plus its SBUF
    traffic slows concurrent PE matmuls.)
  - per-head RMSNorm: sum of squares via the same ones-matmul
    broadcast trick, Ln/Exp on ACT for rsqrt, multiply on DVE.
  - PV:        psum[d, sq]      = vT_tile.T @ probsT
  - out-proj:  psum[s, n]       = oT_slice.T @ w_o_slice; partials are
    stored as fp16 (halves the output DMA traffic; host sums in f64).
Out-proj tiles are emitted one attention tile late so the tensor
engine queue never waits on the normalize chain.  Inputs are staged
host-side in SBUF-matching layouts (hiddens st-major, weights
ko-major) so every DMA row is 8-16KB contiguous -- small-row DMAs run
the rings at <40% of peak and starved the first 40us.  DMA issue is
spread across the three DMA-capable rings (sync/scalar/gpsimd),
ordered just-in-time for the compute sequence.
"""

import sys

sys.path.insert(0, "/opt/trn_rl_repo")

import math
from contextlib import ExitStack

import ml_dtypes
import numpy as np

BF = ml_dtypes.bfloat16

import concourse.bass as bass
import concourse.mybir as mybir
import concourse.tile as tile
from concourse import bacc
from concourse.bass_utils import run_bass_kernel_spmd

F32 = mybir.dt.float32
BF16 = mybir.dt.bfloat16
FP16 = mybir.dt.float16
I32 = mybir.dt.int32
AF = mybir.ActivationFunctionType
OP = mybir.AluOpType

H = 2048          # hidden size
S = 2048          # sequence length
D = 128           # head dim
NH_L = 4          # q heads per rank
QW = NH_L * D     # 512 local q width
CW = QW + 2 * D   # 768 local qkv width
P = 128
KO = H // P       # 16 contraction tiles
ST = S // 512     # 4 seq tiles of 512
SM_SCALE = float(D) ** -0.5
EPS = 1e-6
ROPE_THETA = 10000.0
NEG = -1.0e9


def _build():
    nc = bacc.Bacc("TRN2", target_bir_lowering=False, debug=False, num_devices=8)

    # hiddens st-major and weights ko-major so every DMA row is 8-16KB
    # contiguous (512B-1KB rows run the DMA rings at <40% of peak)
    hT = nc.dram_tensor("hT", [ST * P, KO, 512], BF16,
                        kind="ExternalInput").ap()
    wkv = nc.dram_tensor("wkv", [P, KO, 2 * D], BF16,
                         kind="ExternalInput").ap()
    wqz = nc.dram_tensor("wqz", [P, KO, QW], BF16,
                         kind="ExternalInput").ap()
    wo = nc.dram_tensor("wo", [QW, H], BF16, kind="ExternalInput").ap()
    cosbt = nc.dram_tensor("cosbt", [P, S], BF16, kind="ExternalInput").ap()
    sinbt = nc.dram_tensor("sinbt", [P, S], BF16, kind="ExternalInput").ap()
    wqn = nc.dram_tensor("wqn", [D, 1], F32, kind="ExternalInput").ap()
    wkn = nc.dram_tensor("wkn", [D, 1], F32, kind="ExternalInput").ap()
    maskneg = nc.dram_tensor("maskneg", [P, P], BF16, kind="ExternalInput").ap()
    identr = nc.dram_tensor("identr", [P, P], BF16, kind="ExternalInput").ap()
    rmat = nc.dram_tensor("rmat", [P, P], BF16, kind="ExternalInput").ap()
    out = nc.dram_tensor("out", [4 * ST * P, H], FP16,
                         kind="ExternalOutput").ap()

    hT4 = hT.rearrange("(st p) ko s -> p st ko s", p=P)
    wo3 = wo.rearrange("(ks p) n -> p ks n", p=P)
    out4 = out.rearrange("(t p) n -> p t n", p=P)

    with ExitStack() as ctx:
        tc = ctx.enter_context(tile.TileContext(nc))
        consts = ctx.enter_context(tc.tile_pool(name="consts", bufs=1))
        qkvp = ctx.enter_context(tc.tile_pool(name="qkvp", bufs=1))
        vtp = ctx.enter_context(tc.tile_pool(name="vtp", bufs=1))
        abp = ExitStack()
        csp = abp.enter_context(tc.tile_pool(name="csp", bufs=1))

        # const tiles are allocated here but their (small, low-priority)
        # DMAs are issued inside the st0 block AFTER the critical
        # hidden/weight stream so they don't delay the first matmuls.
        # cos/sin rope tables are host-computed (exact) and DMA'd in
        # per-st chunks in need-order.
        wqn_sb = consts.tile([D, 1], F32)
        wkn_sb = consts.tile([D, 1], F32)
        rmat_sb = consts.tile([P, P], BF16)
        identr_sb = consts.tile([P, P], BF16)
        maskneg_sb = consts.tile([P, P], BF16)
        cosb = csp.tile([P, S], BF16)
        sinb = csp.tile([P, S], BF16)
        ones_sb = consts.tile([P, P], BF16)
        nc.vector.memset(ones_sb, 1.0)
        eps_sb = consts.tile([P, 1], F32)
        nc.vector.memset(eps_sb, EPS)

        # fine-grained tiles (per head / per seq-tile) so readers only
        # wait on the exact producer, not the whole buffer's last writer
        q_sb = [[qkvp.tile([P, 512], BF16, name=f"q_{h}_{s}")
                 for s in range(ST)] for h in range(NH_L)]
        k_sb = [qkvp.tile([P, 512], BF16, name=f"k_{s}") for s in range(ST)]
        vT_sb = [vtp.tile([P, 4, P], BF16, name=f"vT_{s}")
                 for s in range(ST)]

        wqkv_p = abp.enter_context(tc.tile_pool(name="wqkv_p", bufs=1))
        ht_p = abp.enter_context(tc.tile_pool(name="ht_p", bufs=2))
        cpool = abp.enter_context(tc.tile_pool(name="cpool", bufs=1))

        # ---- Phase A: QKV projection + fused norm/rope/v-transpose ----
        # host layout: wqkv cols = [k(128), v(128), q(512)]
        CT_ORDER = [0, 1, 2, 3, 4, 5]  # k, v, then q heads

        def chunk_stage1(x_ch, w_sb, st, ps_c, ps_q, d2):
            """sum-of-squares via full-width ones-matmul into a broadcast
            [128,512] psum (every partition = ssq), rsqrt via Ln/Exp on
            ACT, then the rope rotation.  Emitted one QKV group late so
            the matmuls' DVE inputs are ready when PE reaches them."""
            sl = slice(512 * st, 512 * (st + 1))
            xsq = cpool.tile([P, 512], BF16, tag="ctmp", bufs=4, name="xsq")
            nc.vector.tensor_mul(xsq, x_ch, x_ch)
            sq_ps = ps_q.tile([P, 512], F32, tag="sq", name="sq_ps")
            nc.tensor.matmul(sq_ps, ones_sb, xsq, start=True, stop=True)
            ln_t = cpool.tile([P, 512], F32, tag="lnt", bufs=2, name="ln_t")
            nc.scalar.activation(ln_t, sq_ps, AF.Ln, bias=eps_sb,
                                 scale=1.0 / D)
            rb = cpool.tile([P, 512], BF16, tag="rb", bufs=3, name="rb")
            nc.scalar.activation(rb, ln_t, AF.Exp, scale=-0.5)
            nc.vector.tensor_scalar_mul(x_ch, x_ch, w_sb)
            t1m = cpool.tile([P, 512], BF16, tag="ctmp", bufs=4, name="t1m")
            nc.vector.tensor_mul(t1m, x_ch, cosb[:, sl])
            qr_ps = ps_c.tile([P, 512], F32, tag="qr", name="qr_ps")
            nc.tensor.matmul(qr_ps, rmat_sb, x_ch, start=True, stop=True)
            nc.vector.tensor_tensor(x_ch, qr_ps, sinb[:, sl], OP.mult)
            nc.vector.tensor_tensor(x_ch, x_ch, t1m, OP.add)

            def s2(x_ch=x_ch, rb=rb):
                nc.vector.tensor_tensor(x_ch, x_ch, rb, OP.mult)
            d2.append(s2)

        with nc.named_scope("qkv_proj"):
            with tc.tile_pool(name="ps_a", bufs=4, space="PSUM") as ps_a, \
                 tc.tile_pool(name="ps_c", bufs=2, space="PSUM") as ps_c, \
                 tc.tile_pool(name="ps_q", bufs=2, space="PSUM") as ps_q:
                wkv_sb = wqkv_p.tile([P, KO, 2 * D], BF16)
                wqz_sb = wqkv_p.tile([P, KO, QW], BF16)
                d1 = []  # stage-1 closures, one group late
                d2 = []  # stage-2 closures, drained one per group
                hts = [ht_p.tile([P, KO, 512], BF16, tag="ht",
                                 name=f"ht_{st}") for st in range(ST)]
                for st in range(ST):
                    ssl = slice(512 * st, 512 * (st + 1))
                    ht_sb = hts[st]
                    if st == 0:
                        # st0's critical stream is split across three DMA
                        # rings (~180GB/s each) pulling from HBM in
                        # parallel, ordered just-in-time for the
                        # k,v,q0..q3 psum group sequence; st1's hiddens
                        # are prefetched on whichever rings free up first.
                        # 2-ko first chunks (2KB rows) for a fast start,
                        # then 4-ko (4KB rows) for bandwidth: finer splits
                        # drop to 1KB rows and halve ring bandwidth
                        nc.sync.dma_start(wkv_sb[:, 0:2], wkv[:, 0:2])
                        nc.scalar.dma_start(ht_sb[:, 0:2], hT4[:, st, 0:2])
                        nc.gpsimd.dma_start(ht_sb[:, 2:6], hT4[:, st, 2:6])
                        nc.sync.dma_start(wkv_sb[:, 2:6], wkv[:, 2:6])
                        nc.scalar.dma_start(ht_sb[:, 6:10], hT4[:, st, 6:10])
                        nc.gpsimd.dma_start(ht_sb[:, 10:13], hT4[:, st, 10:13])
                        nc.sync.dma_start(wkv_sb[:, 6:KO], wkv[:, 6:KO])
                        nc.scalar.dma_start(ht_sb[:, 13:16], hT4[:, st, 13:16])
                        nc.sync.dma_start(wqz_sb[:, 0:4], wqz[:, 0:4])
                        nc.gpsimd.dma_start(wqz_sb[:, 4:8], wqz[:, 4:8])
                        nc.sync.dma_start(wqz_sb[:, 8:12], wqz[:, 8:12])
                        nc.gpsimd.dma_start(wqz_sb[:, 12:KO], wqz[:, 12:KO])
                        # rope tables + small consts trail st0's hiddens on
                        # the scalar ring
                        nc.scalar.dma_start(wqn_sb, wqn)
                        nc.scalar.dma_start(wkn_sb, wkn)
                        nc.scalar.dma_start(rmat_sb, rmat)
                        nc.scalar.dma_start(cosb[:, 0:512], cosbt[:, 0:512])
                        nc.scalar.dma_start(sinb[:, 0:512], sinbt[:, 0:512])
                        nc.scalar.dma_start(identr_sb, identr)
                        nc.scalar.dma_start(maskneg_sb, maskneg)
                        # st1 hidden prefetch rides the two rings that
                        # drain first
                        nc.sync.dma_start(hts[1][:, 0:8], hT4[:, 1, 0:8])
                        nc.scalar.dma_start(hts[1][:, 8:KO], hT4[:, 1, 8:KO])
                        nc.gpsimd.dma_start(cosb[:, 512:1024],
                                            cosbt[:, 512:1024])
                        nc.gpsimd.dma_start(sinb[:, 512:1024],
                                            sinbt[:, 512:1024])
                    elif st >= 2:
                        # later hiddens + their rope tables ride the gpsimd
                        # queue in need-order
                        nc.gpsimd.dma_start(ht_sb, hT4[:, st])
                        nc.gpsimd.dma_start(cosb[:, ssl], cosbt[:, ssl])
                        nc.gpsimd.dma_start(sinb[:, ssl], sinbt[:, ssl])
                    for ct in CT_ORDER:
                        acc = ps_a.tile([P, 512], F32, tag="qkv_ps",
                                        name=f"qkv_ps_{st}_{ct}")
                        for ko in range(KO):
                            if ct < 2:
                                wsl = wkv_sb[:, ko, P * ct : P * (ct + 1)]
                            else:
                                wsl = wqz_sb[:, ko,
                                             P * (ct - 2) : P * (ct - 1)]
                            nc.tensor.matmul(
                                acc,
                                wsl,
                                ht_sb[:, ko],
                                start=(ko == 0),
                                stop=(ko == KO - 1),
                            )
                        # psum->sbuf copies run on ACT (idle here), keeping
                        # DVE for the rope/rms elementwise chain
                        if ct == 0:
                            x_ch, w_sb = k_sb[st], wkn_sb
                            nc.scalar.copy(x_ch, acc)
                        elif ct == 1:
                            vch = cpool.tile([P, 512], BF16, tag="vch", bufs=2,
                                             name="vch")
                            nc.scalar.copy(vch, acc)
                        else:
                            x_ch, w_sb = q_sb[ct - 2][st], wqn_sb
                            nc.scalar.copy(x_ch, acc)

                        # run pipelined stages of earlier chunks
                        if d1:
                            d1.pop(0)()
                        if d2:
                            d2.pop(0)()

                        if ct == 1:
                            def vtrans(vch=vch, st=st):
                                for i in range(4):
                                    vt_ps = ps_c.tile([P, P], BF16, tag="qr",
                                                      name="vt_ps")
                                    nc.tensor.transpose(
                                        vt_ps, vch[:, P * i : P * (i + 1)],
                                        identr_sb)
                                    nc.vector.tensor_copy(
                                        vT_sb[st][:, i], vt_ps)
                            d1.append(vtrans)
                        else:
                            def s1(x_ch=x_ch, w_sb=w_sb, st=st):
                                chunk_stage1(x_ch, w_sb, st, ps_c, ps_q, d2)
                            d1.append(s1)
                while d1:
                    d1.pop(0)()
                while d2:
                    d2.pop(0)()

        abp.close()  # release cos/sin tables + norm temps

        # ------------- Phase E/F: attention + out projection -------------
        with tc.tile_pool(name="otp", bufs=1) as otp, \
             tc.tile_pool(name="wop", bufs=1) as wop, \
             tc.tile_pool(name="expp", bufs=12) as expp, \
             tc.tile_pool(name="prp", bufs=16) as prp, \
             tc.tile_pool(name="dip", bufs=2) as dip, \
             tc.tile_pool(name="outp", bufs=3) as outp, \
             tc.tile_pool(name="ps_st", bufs=2, space="PSUM") as ps_st, \
             tc.tile_pool(name="ps_o", bufs=2, space="PSUM") as ps_o, \
             tc.tile_pool(name="ps_dn", bufs=2, space="PSUM") as ps_dn:
            oT_sb = [otp.tile([P, S], BF16, name=f"oT_{h}")
                     for h in range(NH_L)]
            wo_sb = wop.tile([P, NH_L, H], BF16)
            for ks in range(NH_L):
                nc.sync.dma_start(wo_sb[:, ks], wo3[:, ks])

            carry = []  # deferred pv/den/out-proj work from previous heads

            def attn_tile(st, hh):
                n_sk = 4 * st + 4
                qh = q_sb[hh][st]
                o_ps = ps_o.tile([P, 512], F32, tag="o_ps", name="o_ps")
                den_ps = ps_dn.tile([P, 512], F32, tag="dn", name="den_ps")

                def vis0(j):
                    # first visible sq column of sk-block j within this
                    # 512-wide sq tile; columns below it are fully masked
                    # and never computed/read anywhere
                    return max(0, 128 * (j - 4 * st))

                def emit_den(eps):
                    # st0-only: denominator via full-width ones-matmul
                    # broadcast of each exp tile's partition sum into every
                    # psum partition (full PE rate, unlike small-M
                    # one-hots).  For st>0 the exp tiles are tree-summed on
                    # DVE instead and contracted with a single matmul per
                    # head (see below).
                    for ep, j0, _pr in eps:
                        for u in (0, 1):
                            j = j0 + u
                            c0 = vis0(j)
                            nc.tensor.matmul(den_ps[:, c0:512], ones_sb,
                                             ep[:, 512 * u + c0 :
                                                 512 * u + 512],
                                             start=(j == 0),
                                             stop=(j == n_sk - 1))

                def emit_pv(eps):
                    for ep, j0, _pr in eps:
                        for u in (0, 1):
                            j = j0 + u
                            c0 = vis0(j)
                            nc.tensor.matmul(o_ps[:, c0:512],
                                             vT_sb[j // 4][:, j % 4],
                                             ep[:, 512 * u + c0 :
                                                 512 * u + 512],
                                             start=(j == 0),
                                             stop=(j == n_sk - 1))

                pend = []
                prs = []
                epl = []
                quads = []
                for m in range(2 * st + 2):
                    sT = ps_st.tile([P, 1024], F32, tag="sT", name="sT")
                    for u in (0, 1):
                        j = 2 * m + u
                        half = sT[:, 512 * u : 512 * (u + 1)]
                        if j >= 4 * st:
                            # scores over the visible span (start marks the
                            # whole bank), then the [128,128] staircase mask
                            # accumulated on the diagonal block only
                            c0 = vis0(j)
                            nc.tensor.matmul(
                                half[:, c0:512],
                                k_sb[j // 4][:, P * (j % 4) : P * (j % 4 + 1)],
                                qh[:, c0:512],
                                start=True, stop=False)
                            nc.tensor.matmul(
                                half[:, c0 : c0 + 128], identr_sb,
                                maskneg_sb,
                                start=False, stop=True)
                        else:
                            nc.tensor.matmul(
                                half,
                                k_sb[j // 4][:, P * (j % 4) : P * (j % 4 + 1)],
                                qh,
                                start=True, stop=True)
                    ep = expp.tile([P, 1024], BF16, tag="ep", name="ep")
                    # the final staircase tile's first 256 columns are
                    # never read; skip their exp
                    lo = 256 if m == 2 * st + 1 else 0
                    nc.scalar.activation(ep[:, lo:1024], sT[:, lo:1024],
                                         AF.Exp, scale=SM_SCALE)
                    # DVE pre-sum of the tile's two halves for the
                    # denominator (skipped at st0 where every block is on
                    # the staircase and the raw path is already cheap)
                    if st > 0:
                        c1 = vis0(2 * m + 1)
                        pr = prp.tile([P, 512], BF16, tag="pr", name="pr")
                        nc.vector.tensor_tensor(pr[:, c1:512],
                                                ep[:, c1:512],
                                                ep[:, 512 + c1 : 1024],
                                                OP.add)
                    else:
                        pr = None
                    # interleave the previous head's deferred den/PV (its
                    # exps are long done) with this head's scores, so PE
                    # never drains a tile's den/PV right after its exps
                    pend.append((ep, 2 * m, pr))
                    prs.append(pr)
                    epl.append(ep)
                    # fold pair m-1 + pair m early (spreads the DVE tree
                    # work across the m loop instead of bursting at the
                    # head boundary)
                    if st > 0 and m % 2 == 1 and m < 2 * st:
                        qd = prp.tile([P, 512], BF16, tag="pr", name="qd")
                        nc.vector.tensor_tensor(qd, prs[m - 1], prs[m],
                                                OP.add)
                        quads.append(qd)
                    if carry:
                        carry.pop(0)()
                    elif len(pend) > 3:
                        batch, pend = pend[:2], pend[2:]
                        if st == 0:
                            emit_den(batch)
                        emit_pv(batch)

                def mk(batch):
                    def go():
                        if st == 0:
                            emit_den(batch)
                        emit_pv(batch)
                    return go

                while pend:
                    batch, pend = pend[:2], pend[2:]
                    carry.append(mk(batch))

                osl = oT_sb[hh][:, 512 * st : 512 * (st + 1)]

                dsum = None
                if st > 0:
                    # tree-sum the pair tiles on DVE (depth <= 4 bf16
                    # roundings) so the head's whole denominator costs a
                    # single full-width PE matmul
                    lvl = quads if quads else prs[: 2 * st]
                    while len(lvl) > 1:
                        nxt = []
                        for i in range(0, len(lvl) - 1, 2):
                            tr = prp.tile([P, 512], BF16, tag="pr",
                                          name="tr")
                            nc.vector.tensor_tensor(tr, lvl[i], lvl[i + 1],
                                                    OP.add)
                            nxt.append(tr)
                        if len(lvl) % 2:
                            nxt.append(lvl[-1])
                        lvl = nxt
                    dsum = lvl[0]
                    # fold in the two staircase tiles' visible regions
                    nc.vector.tensor_tensor(
                        dsum[:, 128:512], dsum[:, 128:512],
                        prs[2 * st][:, 128:512], OP.add)
                    nc.vector.tensor_tensor(
                        dsum[:, 0:128], dsum[:, 0:128],
                        epl[2 * st][:, 0:128], OP.add)
                    nc.vector.tensor_tensor(
                        dsum[:, 384:512], dsum[:, 384:512],
                        prs[2 * st + 1][:, 384:512], OP.add)
                    nc.vector.tensor_tensor(
                        dsum[:, 256:384], dsum[:, 256:384],
                        epl[2 * st + 1][:, 256:384], OP.add)

                def fin_head(den_ps=den_ps, o_ps=o_ps, osl=osl, dsum=dsum):
                    # 1/den on DVE (18-bit custom op), normalize fused into
                    # the PV psum->sbuf copy
                    if dsum is not None:
                        nc.tensor.matmul(den_ps, ones_sb, dsum,
                                         start=True, stop=True)
                    di = dip.tile([P, 512], F32, tag="di", name="di")
                    nc.vector.reciprocal_approx_fast(di, den_ps)
                    nc.vector.tensor_tensor(osl, o_ps, di, OP.mult)
                carry.append(fin_head)
                return osl

            def out_proj(st):
                # the final tile's out-proj is the serial tail: attention is
                # done, so spread its psum groups over the idle scores slots
                # and its copies over both ACT and DVE; the tail's stores go
                # out in halves on alternating queues so the last DMAs don't
                # all trail the final copy
                tail = st == ST - 1
                with nc.named_scope(f"out_proj_t{st}"):
                    for ti in range(4):
                        t = 4 * st + ti
                        ob = outp.tile([P, H], FP16, tag="out_sb",
                                       name="out_sb")
                        for nt in range(4):
                            if tail and nt % 2 == 1:
                                acc = ps_st.tile([P, 512], F32, tag="sT",
                                                 name="out_ps_b")
                            else:
                                acc = ps_o.tile([P, 512], F32, tag="o_ps",
                                                name="out_ps")
                            for ks in range(NH_L):
                                nc.tensor.matmul(
                                    acc,
                                    oT_sb[ks][:, P * t : P * (t + 1)],
                                    wo_sb[:, ks, 512 * nt : 512 * (nt + 1)],
                                    start=(ks == 0),
                                    stop=(ks == NH_L - 1),
                                )
                            dst = ob[:, 512 * nt : 512 * (nt + 1)]
                            # all out copies on ACT: DVE casts reading psum
                            # were stalling concurrent matmul psum writes
                            # (~380ns vs 216ns), and ACT has phase-B slack
                            nc.scalar.copy(dst, acc)
                            if tail and nt % 2 == 1:
                                q = nc.sync if nt == 1 else nc.scalar
                                q.dma_start(
                                    out4[:, t, 512 * (nt - 1) : 512 * (nt + 1)],
                                    ob[:, 512 * (nt - 1) : 512 * (nt + 1)])
                        if not tail:
                            nc.sync.dma_start(out4[:, t], ob)

            for st in range(ST):
                for hh in range(NH_L):
                    with nc.named_scope(f"attn_h{hh}_t{st}"):
                        attn_tile(st, hh)

                def fin_st(st=st):
                    out_proj(st)
                carry.append(fin_st)
            while carry:
                carry.pop(0)()

    nc.compile()
    _merge_act_table_loads(nc)
    return nc


def _merge_act_table_loads(nc):
    """Ln(5)/Exp+Copy(0) both live in set 6 (natural_log_exp_and_others);
    bass's per-function table choice alternates 5/0 and reloads tables at
    every Ln<->Exp transition (~1.5us each).  Retarget those loads to
    set 6 and drop the now-redundant reloads."""
    for b in nc.main_func.blocks:
        loaded = None
        keep = []
        for inst in b.instructions:
            if isinstance(inst, mybir.InstLoadActFuncSet):
                tid = inst.act_func_set_id
                if tid in (0, 5):
                    tid = 6
                if tid == loaded:
                    continue
                inst.act_func_set_id = tid
                loaded = tid
            keep.append(inst)
        b.instructions[:] = keep


_NC_CACHE = None


def _get_nc():
    global _NC_CACHE
    if _NC_CACHE is None:
        _NC_CACHE = _build()
    return _NC_CACHE


def _host_inputs(positions, hidden_states, w_qkv, w_o, q_norm_w, k_norm_w):
    """Build the 8 per-core input maps."""
    positions = np.asarray(positions, dtype=np.int32)
    hidden_states = np.asarray(hidden_states, dtype=np.float32)
    w_qkv = np.asarray(w_qkv, dtype=np.float32)
    w_o = np.asarray(w_o, dtype=np.float32)
    q_norm_w = np.asarray(q_norm_w, dtype=np.float32)
    k_norm_w = np.asarray(k_norm_w, dtype=np.float32)

    invf = 1.0 / (ROPE_THETA ** (np.arange(0, D, 2, dtype=np.float64) / D))
    p_idx = np.arange(P).reshape(P, 1)
    c_idx = np.arange(P).reshape(1, P)
    maskneg = np.where(p_idx > c_idx, np.float32(NEG), np.float32(0.0))
    maskneg = maskneg.astype(BF)
    identr = np.eye(P, dtype=BF)
    rmat = np.zeros((P, P), dtype=BF)
    for i in range(64):
        rmat[64 + i, i] = -1.0
        rmat[i, 64 + i] = 1.0
    wqn = q_norm_w.reshape(D, 1)
    wkn = k_norm_w.reshape(D, 1)

    # host-exact rope tables per batch: row r (r%64 = freq) x position
    cosbt, sinbt = [], []
    for g in range(positions.shape[0]):
        ang = np.outer(invf, positions[g].astype(np.float64))  # [64, S]
        c = np.cos(ang).astype(BF)
        s = np.sin(ang).astype(BF)
        cosbt.append(np.concatenate([c, c], axis=0))
        sinbt.append(np.concatenate([s, s], axis=0))

    # hiddens st-major [ST*P, KO, 512] so each per-st DMA has 16KB
    # contiguous rows
    hT_st = []
    for g in range(hidden_states.shape[0]):
        a = hidden_states[g].T.reshape(KO, P, ST, 512)
        hT_st.append(
            np.ascontiguousarray(a.transpose(2, 1, 0, 3))
            .reshape(ST * P, KO, 512).astype(BF))

    in_maps = []
    for core in range(8):
        g, r = core // 4, core % 4
        wq_cols = w_qkv[:, 512 * r : 512 * (r + 1)]
        wk_col = w_qkv[:, 2048 + 128 * r : 2048 + 128 * (r + 1)]
        wv_col = w_qkv[:, 2560 + 128 * r : 2560 + 128 * (r + 1)]
        # weights ko-major [P, KO, cols]: 8-16KB contiguous DMA rows
        wkv_h = np.concatenate([wk_col, wv_col], axis=1)  # [H, 256]
        wkv_h = np.ascontiguousarray(
            wkv_h.reshape(KO, P, 2 * D).transpose(1, 0, 2)).astype(BF)
        wqz_h = np.ascontiguousarray(
            wq_cols.reshape(KO, P, QW).transpose(1, 0, 2)).astype(BF)
        in_maps.append(
            {
                "hT": hT_st[g],
                "wkv": wkv_h,
                "wqz": wqz_h,
                "wo": np.ascontiguousarray(
                    w_o[512 * r : 512 * (r + 1), :]
                ).astype(BF),
                "cosbt": cosbt[g],
                "sinbt": sinbt[g],
                "wqn": wqn,
                "wkn": wkn,
                "maskneg": maskneg,
                "identr": identr,
                "rmat": rmat,
            }
        )
    return in_maps


def run(trace=False, **inputs):
    nc = _get_nc()
    in_maps = _host_inputs(**inputs)
    res = run_bass_kernel_spmd(nc, in_maps, core_ids=list(range(8)), trace=trace)
    B = inputs["hidden_states"].shape[0]
    out = np.zeros((B, S, H), dtype=np.float64)
    for core in range(8):
        g = core // 4
        out[g] += res.results[core]["out"].astype(np.float64)
    return out.astype(np.float32), res


def kernel(**inputs):
    out, _ = run(trace=False, **inputs)
    return out


# revision 42
# speedup vs baseline: 1.0117x; 1.0082x over previous
"""BaiLing attention Trainium2 kernel.

Sharding: 8 cores = 2 (batch) x 4 (tensor-parallel over heads).
Each TP rank r owns q heads 4r..4r+3 and kv head r (GQA group-aligned),
computes its out-projection partial; host sums the 4 partials per batch.

On-chip layout is feature-major [d, s] everywhere:
  - QKV proj:  psum[qkv_col, s] = w_qkv_slice.T @ hidden.T
  - scoresT:   psum[sk, sq]     = k_tile.T @ q_tile    (both [d, *])
  - causal mask: -1e9 accumulated into the scores psum via an
    identity-matmul of a constant mask tile
  - softmax over sk (partition dim): exp on ACT; the denominator is
    built by pair/tree-summing the exp tiles on DVE (bf16, depth<=4
    roundings) and contracting once per (tile,head) with a full-width
    [128,128] ones-matmul into a broadcast [128,512] psum (every
    partition = den).  Small-M one-hot matmuls (the old scheme) run
    ~1.5x slower on PE and stall the following weight load; full-M
    ones matmuls run at peak.  1/den comes from the DVE
    reciprocal_approx_fast custom op (18 significant bits) and the
    normalize is fused into the PSUM->SBUF copy of the PV output.
    (Elementwise denominator accumulation on the GpSimd/Pool engine
    was tried and is ~4x slower than PE/DVE here, plus its SBUF
    traffic slows concurrent PE matmuls.)
  - per-head RMSNorm: sum of squares via the same ones-matmul
    broadcast trick, Ln/Exp on ACT for rsqrt, multiply on DVE.
  - PV:        psum[d, sq]      = vT_tile.T @ probsT
  - out-proj:  psum[s, n]       = oT_slice.T @ w_o_slice; partials are
    stored as fp16 (halves the output DMA traffic; host sums in f64).
Out-proj tiles are emitted one attention tile late so the tensor
engine queue never waits on the normalize chain.  Inputs are staged
host-side in SBUF-matching layouts (hiddens st-major, weights
ko-major) so every DMA row is 8-16KB contiguous -- small-row DMAs run
the rings at <40% of peak and starved the first 40us.  DMA issue is
spread across the three DMA-capable rings (sync/scalar/gpsimd),
ordered just-in-time for the compute sequence.
"""

import sys

sys.path.insert(0, "/opt/trn_rl_repo")

import math
from contextlib import ExitStack

import ml_dtypes
import numpy as np

BF = ml_dtypes.bfloat16

import concourse.bass as bass
import concourse.mybir as mybir
import concourse.tile as tile
from concourse import bacc
from concourse.bass_utils import run_bass_kernel_spmd

F32 = mybir.dt.float32
BF16 = mybir.dt.bfloat16
FP16 = mybir.dt.float16
I32 = mybir.dt.int32
AF = mybir.ActivationFunctionType
OP = mybir.AluOpType

H = 2048          # hidden size
S = 2048          # sequence length
D = 128           # head dim
NH_L = 4          # q heads per rank
QW = NH_L * D     # 512 local q width
CW = QW + 2 * D   # 768 local qkv width
P = 128
KO = H // P       # 16 contraction tiles
ST = S // 512     # 4 seq tiles of 512
SM_SCALE = float(D) ** -0.5
EPS = 1e-6
ROPE_THETA = 10000.0
NEG = -1.0e9


def _build():
    nc = bacc.Bacc("TRN2", target_bir_lowering=False, debug=False, num_devices=8)

    # hiddens st-major and weights ko-major so every DMA row is 8-16KB
    # contiguous (512B-1KB rows run the DMA rings at <40% of peak)
    hT = nc.dram_tensor("hT", [ST * P, KO, 512], BF16,
                        kind="ExternalInput").ap()
    wkv = nc.dram_tensor("wkv", [P, KO, 2 * D], BF16,
                         kind="ExternalInput").ap()
    wqz = nc.dram_tensor("wqz", [P, KO, QW], BF16,
                         kind="ExternalInput").ap()
    wo = nc.dram_tensor("wo", [QW, H], BF16, kind="ExternalInput").ap()
    cosbt = nc.dram_tensor("cosbt", [P, S], BF16, kind="ExternalInput").ap()
    sinbt = nc.dram_tensor("sinbt", [P, S], BF16, kind="ExternalInput").ap()
    wqn = nc.dram_tensor("wqn", [D, 1], F32, kind="ExternalInput").ap()
    wkn = nc.dram_tensor("wkn", [D, 1], F32, kind="ExternalInput").ap()
    maskneg = nc.dram_tensor("maskneg", [P, P], BF16, kind="ExternalInput").ap()
    identr = nc.dram_tensor("identr", [P, P], BF16, kind="ExternalInput").ap()
    rmat = nc.dram_tensor("rmat", [P, P], BF16, kind="ExternalInput").ap()
    out = nc.dram_tensor("out", [4 * ST * P, H], FP16,
                         kind="ExternalOutput").ap()

    hT4 = hT.rearrange("(st p) ko s -> p st ko s", p=P)
    wo3 = wo.rearrange("(ks p) n -> p ks n", p=P)
    out4 = out.rearrange("(t p) n -> p t n", p=P)

    with ExitStack() as ctx:
        tc = ctx.enter_context(tile.TileContext(nc))
        consts = ctx.enter_context(tc.tile_pool(name="consts", bufs=1))
        qkvp = ctx.enter_context(tc.tile_pool(name="qkvp", bufs=1))
        vtp = ctx.enter_context(tc.tile_pool(name="vtp", bufs=1))
        abp = ExitStack()
        csp = abp.enter_context(tc.tile_pool(name="csp", bufs=1))

        # const tiles are allocated here but their (small, low-priority)
        # DMAs are issued inside the st0 block AFTER the critical
        # hidden/weight stream so they don't delay the first matmuls.
        # cos/sin rope tables are host-computed (exact) and DMA'd in
        # per-st chunks in need-order.
        wqn_sb = consts.tile([D, 1], F32)
        wkn_sb = consts.tile([D, 1], F32)
        rmat_sb = consts.tile([P, P], BF16)
        identr_sb = consts.tile([P, P], BF16)
        maskneg_sb = consts.tile([P, P], BF16)
        cosb = csp.tile([P, S], BF16)
        sinb = csp.tile([P, S], BF16)
        ones_sb = consts.tile([P, P], BF16)
        nc.vector.memset(ones_sb, 1.0)
        eps_sb = consts.tile([P, 1], F32)
        nc.vector.memset(eps_sb, EPS)

        # fine-grained tiles (per head / per seq-tile) so readers only
        # wait on the exact producer, not the whole buffer's last writer
        q_sb = [[qkvp.tile([P, 512], BF16, name=f"q_{h}_{s}")
                 for s in range(ST)] for h in range(NH_L)]
        k_sb = [qkvp.tile([P, 512], BF16, name=f"k_{s}") for s in range(ST)]
        vT_sb = [vtp.tile([P, 4, P], BF16, name=f"vT_{s}")
                 for s in range(ST)]

        wqkv_p = abp.enter_context(tc.tile_pool(name="wqkv_p", bufs=1))
        ht_p = abp.enter_context(tc.tile_pool(name="ht_p", bufs=2))
        cpool = abp.enter_context(tc.tile_pool(name="cpool", bufs=1))

        # ---- Phase A: QKV projection + fused norm/rope/v-transpose ----
        # host layout: wqkv cols = [k(128), v(128), q(512)]
        CT_ORDER = [0, 1, 2, 3, 4, 5]  # k, v, then q heads

        def chunk_stage1(x_ch, w_sb, st, ps_c, ps_q, d2):
            """sum-of-squares via full-width ones-matmul into a broadcast
            [128,512] psum (every partition = ssq), rsqrt via Ln/Exp on
            ACT, then the rope rotation.  Emitted one QKV group late so
            the matmuls' DVE inputs are ready when PE reaches them."""
            sl = slice(512 * st, 512 * (st + 1))
            xsq = cpool.tile([P, 512], BF16, tag="ctmp", bufs=4, name="xsq")
            nc.vector.tensor_mul(xsq, x_ch, x_ch)
            sq_ps = ps_q.tile([P, 512], F32, tag="sq", name="sq_ps")
            nc.tensor.matmul(sq_ps, ones_sb, xsq, start=True, stop=True)
            ln_t = cpool.tile([P, 512], F32, tag="lnt", bufs=2, name="ln_t")
            nc.scalar.activation(ln_t, sq_ps, AF.Ln, bias=eps_sb,
                                 scale=1.0 / D)
            rb = cpool.tile([P, 512], BF16, tag="rb", bufs=3, name="rb")
            nc.scalar.activation(rb, ln_t, AF.Exp, scale=-0.5)
            nc.vector.tensor_scalar_mul(x_ch, x_ch, w_sb)
            t1m = cpool.tile([P, 512], BF16, tag="ctmp", bufs=4, name="t1m")
            nc.vector.tensor_mul(t1m, x_ch, cosb[:, sl])
            qr_ps = ps_c.tile([P, 512], F32, tag="qr", name="qr_ps")
            nc.tensor.matmul(qr_ps, rmat_sb, x_ch, start=True, stop=True)
            nc.vector.tensor_tensor(x_ch, qr_ps, sinb[:, sl], OP.mult)
            nc.vector.tensor_tensor(x_ch, x_ch, t1m, OP.add)

            def s2(x_ch=x_ch, rb=rb):
                nc.vector.tensor_tensor(x_ch, x_ch, rb, OP.mult)
            d2.append(s2)

        with nc.named_scope("qkv_proj"):
            with tc.tile_pool(name="ps_a", bufs=4, space="PSUM") as ps_a, \
                 tc.tile_pool(name="ps_c", bufs=2, space="PSUM") as ps_c, \
                 tc.tile_pool(name="ps_q", bufs=2, space="PSUM") as ps_q:
                wkv_sb = wqkv_p.tile([P, KO, 2 * D], BF16)
                wqz_sb = wqkv_p.tile([P, KO, QW], BF16)
                d1 = []  # stage-1 closures, one group late
                d2 = []  # stage-2 closures, drained one per group
                hts = [ht_p.tile([P, KO, 512], BF16, tag="ht",
                                 name=f"ht_{st}") for st in range(ST)]
                for st in range(ST):
                    ssl = slice(512 * st, 512 * (st + 1))
                    ht_sb = hts[st]
                    if st == 0:
                        # st0's critical stream is split across three DMA
                        # rings (~180GB/s each) pulling from HBM in
                        # parallel, ordered just-in-time for the
                        # k,v,q0..q3 psum group sequence; st1's hiddens
                        # are prefetched on whichever rings free up first.
                        # 2-ko first chunks (2KB rows) for a fast start,
                        # then 4-ko (4KB rows) for bandwidth: finer splits
                        # drop to 1KB rows and halve ring bandwidth
                        nc.sync.dma_start(wkv_sb[:, 0:2], wkv[:, 0:2])
                        nc.scalar.dma_start(ht_sb[:, 0:2], hT4[:, st, 0:2])
                        nc.gpsimd.dma_start(ht_sb[:, 2:6], hT4[:, st, 2:6])
                        nc.sync.dma_start(wkv_sb[:, 2:6], wkv[:, 2:6])
                        nc.scalar.dma_start(ht_sb[:, 6:10], hT4[:, st, 6:10])
                        nc.gpsimd.dma_start(ht_sb[:, 10:13], hT4[:, st, 10:13])
                        nc.sync.dma_start(wkv_sb[:, 6:KO], wkv[:, 6:KO])
                        nc.scalar.dma_start(ht_sb[:, 13:16], hT4[:, st, 13:16])
                        nc.sync.dma_start(wqz_sb[:, 0:4], wqz[:, 0:4])
                        nc.gpsimd.dma_start(wqz_sb[:, 4:8], wqz[:, 4:8])
                        nc.sync.dma_start(wqz_sb[:, 8:12], wqz[:, 8:12])
                        nc.gpsimd.dma_start(wqz_sb[:, 12:KO], wqz[:, 12:KO])
                        # rope tables + small consts trail st0's hiddens on
                        # the scalar ring
                        nc.scalar.dma_start(wqn_sb, wqn)
                        nc.scalar.dma_start(wkn_sb, wkn)
                        nc.scalar.dma_start(rmat_sb, rmat)
                        nc.scalar.dma_start(cosb[:, 0:512], cosbt[:, 0:512])
                        nc.scalar.dma_start(sinb[:, 0:512], sinbt[:, 0:512])
                        nc.gpsimd.dma_start(identr_sb, identr)
                        nc.gpsimd.dma_start(maskneg_sb, maskneg)
                        # st1 hidden prefetch rides the two rings that
                        # drain first
                        nc.sync.dma_start(hts[1][:, 0:8], hT4[:, 1, 0:8])
                        nc.scalar.dma_start(hts[1][:, 8:KO], hT4[:, 1, 8:KO])
                        nc.gpsimd.dma_start(cosb[:, 512:1024],
                                            cosbt[:, 512:1024])
                        nc.gpsimd.dma_start(sinb[:, 512:1024],
                                            sinbt[:, 512:1024])
                    elif st >= 2:
                        # later hiddens + their rope tables ride the gpsimd
                        # queue in need-order
                        nc.gpsimd.dma_start(ht_sb, hT4[:, st])
                        nc.gpsimd.dma_start(cosb[:, ssl], cosbt[:, ssl])
                        nc.gpsimd.dma_start(sinb[:, ssl], sinbt[:, ssl])
                    for ct in CT_ORDER:
                        acc = ps_a.tile([P, 512], F32, tag="qkv_ps",
                                        name=f"qkv_ps_{st}_{ct}")
                        for ko in range(KO):
                            if ct < 2:
                                wsl = wkv_sb[:, ko, P * ct : P * (ct + 1)]
                            else:
                                wsl = wqz_sb[:, ko,
                                             P * (ct - 2) : P * (ct - 1)]
                            nc.tensor.matmul(
                                acc,
                                wsl,
                                ht_sb[:, ko],
                                start=(ko == 0),
                                stop=(ko == KO - 1),
                            )
                        # psum->sbuf copies run on ACT (idle here), keeping
                        # DVE for the rope/rms elementwise chain
                        if ct == 0:
                            x_ch, w_sb = k_sb[st], wkn_sb
                            nc.scalar.copy(x_ch, acc)
                        elif ct == 1:
                            vch = cpool.tile([P, 512], BF16, tag="vch", bufs=2,
                                             name="vch")
                            nc.scalar.copy(vch, acc)
                        else:
                            x_ch, w_sb = q_sb[ct - 2][st], wqn_sb
                            nc.scalar.copy(x_ch, acc)

                        # run pipelined stages of earlier chunks; on the
                        # last seq tile drain two per group so no DVE
                        # backlog stalls the attention-phase entry
                        if d1:
                            d1.pop(0)()
                        if d2:
                            d2.pop(0)()
                        if st == ST - 1:
                            if d1:
                                d1.pop(0)()
                            if d2:
                                d2.pop(0)()

                        if ct == 1:
                            def vtrans(vch=vch, st=st):
                                for i in range(4):
                                    vt_ps = ps_c.tile([P, P], BF16, tag="qr",
                                                      name="vt_ps")
                                    nc.tensor.transpose(
                                        vt_ps, vch[:, P * i : P * (i + 1)],
                                        identr_sb)
                                    nc.vector.tensor_copy(
                                        vT_sb[st][:, i], vt_ps)
                            d1.append(vtrans)
                        else:
                            def s1(x_ch=x_ch, w_sb=w_sb, st=st):
                                chunk_stage1(x_ch, w_sb, st, ps_c, ps_q, d2)
                            d1.append(s1)
                while d1:
                    d1.pop(0)()
                while d2:
                    d2.pop(0)()

        abp.close()  # release cos/sin tables + norm temps

        # ------------- Phase E/F: attention + out projection -------------
        with tc.tile_pool(name="otp", bufs=1) as otp, \
             tc.tile_pool(name="wop", bufs=1) as wop, \
             tc.tile_pool(name="expp", bufs=12) as expp, \
             tc.tile_pool(name="prp", bufs=16) as prp, \
             tc.tile_pool(name="dip", bufs=2) as dip, \
             tc.tile_pool(name="outp", bufs=3) as outp, \
             tc.tile_pool(name="ps_st", bufs=2, space="PSUM") as ps_st, \
             tc.tile_pool(name="ps_o", bufs=2, space="PSUM") as ps_o, \
             tc.tile_pool(name="ps_dn", bufs=2, space="PSUM") as ps_dn:
            oT_sb = [otp.tile([P, S], BF16, name=f"oT_{h}")
                     for h in range(NH_L)]
            wo_sb = wop.tile([P, NH_L, H], BF16)
            for ks in range(NH_L):
                nc.sync.dma_start(wo_sb[:, ks], wo3[:, ks])

            carry = []  # deferred pv/den/out-proj work from previous heads

            def attn_tile(st, hh):
                n_sk = 4 * st + 4
                qh = q_sb[hh][st]
                o_ps = ps_o.tile([P, 512], F32, tag="o_ps", name="o_ps")
                den_ps = ps_dn.tile([P, 512], F32, tag="dn", name="den_ps")

                def vis0(j):
                    # first visible sq column of sk-block j within this
                    # 512-wide sq tile; columns below it are fully masked
                    # and never computed/read anywhere
                    return max(0, 128 * (j - 4 * st))

                def emit_den(eps):
                    # st0-only: denominator via full-width ones-matmul
                    # broadcast of each exp tile's partition sum into every
                    # psum partition (full PE rate, unlike small-M
                    # one-hots).  For st>0 the exp tiles are tree-summed on
                    # DVE instead and contracted with a single matmul per
                    # head (see below).
                    for ep, j0, _pr in eps:
                        for u in (0, 1):
                            j = j0 + u
                            c0 = vis0(j)
                            nc.tensor.matmul(den_ps[:, c0:512], ones_sb,
                                             ep[:, 512 * u + c0 :
                                                 512 * u + 512],
                                             start=(j == 0),
                                             stop=(j == n_sk - 1))

                def emit_pv(eps):
                    for ep, j0, _pr in eps:
                        for u in (0, 1):
                            j = j0 + u
                            c0 = vis0(j)
                            nc.tensor.matmul(o_ps[:, c0:512],
                                             vT_sb[j // 4][:, j % 4],
                                             ep[:, 512 * u + c0 :
                                                 512 * u + 512],
                                             start=(j == 0),
                                             stop=(j == n_sk - 1))

                pend = []
                prs = []
                epl = []
                quads = []
                for m in range(2 * st + 2):
                    sT = ps_st.tile([P, 1024], F32, tag="sT", name="sT")
                    for u in (0, 1):
                        j = 2 * m + u
                        half = sT[:, 512 * u : 512 * (u + 1)]
                        if j >= 4 * st:
                            # scores over the visible span (start marks the
                            # whole bank), then the [128,128] staircase mask
                            # accumulated on the diagonal block only
                            c0 = vis0(j)
                            nc.tensor.matmul(
                                half[:, c0:512],
                                k_sb[j // 4][:, P * (j % 4) : P * (j % 4 + 1)],
                                qh[:, c0:512],
                                start=True, stop=False)
                            nc.tensor.matmul(
                                half[:, c0 : c0 + 128], identr_sb,
                                maskneg_sb,
                                start=False, stop=True)
                        else:
                            nc.tensor.matmul(
                                half,
                                k_sb[j // 4][:, P * (j % 4) : P * (j % 4 + 1)],
                                qh,
                                start=True, stop=True)
                    ep = expp.tile([P, 1024], BF16, tag="ep", name="ep")
                    # the final staircase tile's first 256 columns are
                    # never read; skip their exp
                    lo = 256 if m == 2 * st + 1 else 0
                    nc.scalar.activation(ep[:, lo:1024], sT[:, lo:1024],
                                         AF.Exp, scale=SM_SCALE)
                    # DVE pre-sum of the tile's two halves for the
                    # denominator (skipped at st0 where every block is on
                    # the staircase and the raw path is already cheap)
                    if st > 0:
                        c1 = vis0(2 * m + 1)
                        pr = prp.tile([P, 512], BF16, tag="pr", name="pr")
                        nc.vector.tensor_tensor(pr[:, c1:512],
                                                ep[:, c1:512],
                                                ep[:, 512 + c1 : 1024],
                                                OP.add)
                    else:
                        pr = None
                    # interleave the previous head's deferred den/PV (its
                    # exps are long done) with this head's scores, so PE
                    # never drains a tile's den/PV right after its exps
                    pend.append((ep, 2 * m, pr))
                    prs.append(pr)
                    epl.append(ep)
                    # fold pair m-1 + pair m early (spreads the DVE tree
                    # work across the m loop instead of bursting at the
                    # head boundary)
                    if st > 0 and m % 2 == 1 and m < 2 * st:
                        qd = prp.tile([P, 512], BF16, tag="pr", name="qd")
                        nc.vector.tensor_tensor(qd, prs[m - 1], prs[m],
                                                OP.add)
                        quads.append(qd)
                    if carry:
                        carry.pop(0)()
                    elif len(pend) > 3:
                        batch, pend = pend[:2], pend[2:]
                        if st == 0:
                            emit_den(batch)
                        emit_pv(batch)

                def mk(batch):
                    def go():
                        if st == 0:
                            emit_den(batch)
                        emit_pv(batch)
                    return go

                while pend:
                    batch, pend = pend[:2], pend[2:]
                    carry.append(mk(batch))

                osl = oT_sb[hh][:, 512 * st : 512 * (st + 1)]

                dsum = None
                if st > 0:
                    # tree-sum the pair tiles on DVE (depth <= 4 bf16
                    # roundings) so the head's whole denominator costs a
                    # single full-width PE matmul
                    lvl = quads if quads else prs[: 2 * st]
                    while len(lvl) > 1:
                        nxt = []
                        for i in range(0, len(lvl) - 1, 2):
                            tr = prp.tile([P, 512], BF16, tag="pr",
                                          name="tr")
                            nc.vector.tensor_tensor(tr, lvl[i], lvl[i + 1],
                                                    OP.add)
                            nxt.append(tr)
                        if len(lvl) % 2:
                            nxt.append(lvl[-1])
                        lvl = nxt
                    dsum = lvl[0]
                    # fold in the two staircase tiles' visible regions
                    nc.vector.tensor_tensor(
                        dsum[:, 128:512], dsum[:, 128:512],
                        prs[2 * st][:, 128:512], OP.add)
                    nc.vector.tensor_tensor(
                        dsum[:, 0:128], dsum[:, 0:128],
                        epl[2 * st][:, 0:128], OP.add)
                    nc.vector.tensor_tensor(
                        dsum[:, 384:512], dsum[:, 384:512],
                        prs[2 * st + 1][:, 384:512], OP.add)
                    nc.vector.tensor_tensor(
                        dsum[:, 256:384], dsum[:, 256:384],
                        epl[2 * st + 1][:, 256:384], OP.add)

                def fin_head(den_ps=den_ps, o_ps=o_ps, osl=osl, dsum=dsum):
                    # 1/den on DVE (18-bit custom op), normalize fused into
                    # the PV psum->sbuf copy
                    if dsum is not None:
                        nc.tensor.matmul(den_ps, ones_sb, dsum,
                                         start=True, stop=True)
                    di = dip.tile([P, 512], F32, tag="di", name="di")
                    nc.vector.reciprocal_approx_fast(di, den_ps)
                    nc.vector.tensor_tensor(osl, o_ps, di, OP.mult)
                carry.append(fin_head)
                return osl

            def out_proj(st):
                # the final tile's out-proj is the serial tail: attention is
                # done, so spread its psum groups over the idle scores slots
                # and its copies over both ACT and DVE; the tail's stores go
                # out in halves on alternating queues so the last DMAs don't
                # all trail the final copy
                tail = st == ST - 1
                with nc.named_scope(f"out_proj_t{st}"):
                    for ti in range(4):
                        t = 4 * st + ti
                        ob = outp.tile([P, H], FP16, tag="out_sb",
                                       name="out_sb")
                        for nt in range(4):
                            if tail and nt % 2 == 1:
                                acc = ps_st.tile([P, 512], F32, tag="sT",
                                                 name="out_ps_b")
                            else:
                                acc = ps_o.tile([P, 512], F32, tag="o_ps",
                                                name="out_ps")
                            for ks in range(NH_L):
                                nc.tensor.matmul(
                                    acc,
                                    oT_sb[ks][:, P * t : P * (t + 1)],
                                    wo_sb[:, ks, 512 * nt : 512 * (nt + 1)],
                                    start=(ks == 0),
                                    stop=(ks == NH_L - 1),
                                )
                            dst = ob[:, 512 * nt : 512 * (nt + 1)]
                            # all out copies on ACT: DVE casts reading psum
                            # were stalling concurrent matmul psum writes
                            # (~380ns vs 216ns), and ACT has phase-B slack
                            nc.scalar.copy(dst, acc)
                            if tail and nt % 2 == 1:
                                q = nc.sync if nt == 1 else nc.scalar
                                q.dma_start(
                                    out4[:, t, 512 * (nt - 1) : 512 * (nt + 1)],
                                    ob[:, 512 * (nt - 1) : 512 * (nt + 1)])
                        if not tail:
                            nc.sync.dma_start(out4[:, t], ob)

            for st in range(ST):
                for hh in range(NH_L):
                    with nc.named_scope(f"attn_h{hh}_t{st}"):
                        attn_tile(st, hh)

                def fin_st(st=st):
                    out_proj(st)
                carry.append(fin_st)
            while carry:
                carry.pop(0)()

    nc.compile()
    _merge_act_table_loads(nc)
    return nc


def _merge_act_table_loads(nc):
    """Ln(5)/Exp+Copy(0) both live in set 6 (natural_log_exp_and_others);
    bass's per-function table choice alternates 5/0 and reloads tables at
    every Ln<->Exp transition (~1.5us each).  Retarget those loads to
    set 6 and drop the now-redundant reloads."""
    for b in nc.main_func.blocks:
        loaded = None
        keep = []
        for inst in b.instructions:
            if isinstance(inst, mybir.InstLoadActFuncSet):
                tid = inst.act_func_set_id
                if tid in (0, 5):
                    tid = 6
                if tid == loaded:
                    continue
                inst.act_func_set_id = tid
                loaded = tid
            keep.append(inst)
        b.instructions[:] = keep


_NC_CACHE = None


def _get_nc():
    global _NC_CACHE
    if _NC_CACHE is None:
        _NC_CACHE = _build()
    return _NC_CACHE


def _host_inputs(positions, hidden_states, w_qkv, w_o, q_norm_w, k_norm_w):
    """Build the 8 per-core input maps."""
    positions = np.asarray(positions, dtype=np.int32)
    hidden_states = np.asarray(hidden_states, dtype=np.float32)
    w_qkv = np.asarray(w_qkv, dtype=np.float32)
    w_o = np.asarray(w_o, dtype=np.float32)
    q_norm_w = np.asarray(q_norm_w, dtype=np.float32)
    k_norm_w = np.asarray(k_norm_w, dtype=np.float32)

    invf = 1.0 / (ROPE_THETA ** (np.arange(0, D, 2, dtype=np.float64) / D))
    p_idx = np.arange(P).reshape(P, 1)
    c_idx = np.arange(P).reshape(1, P)
    maskneg = np.where(p_idx > c_idx, np.float32(NEG), np.float32(0.0))
    maskneg = maskneg.astype(BF)
    identr = np.eye(P, dtype=BF)
    rmat = np.zeros((P, P), dtype=BF)
    for i in range(64):
        rmat[64 + i, i] = -1.0
        rmat[i, 64 + i] = 1.0
    wqn = q_norm_w.reshape(D, 1)
    wkn = k_norm_w.reshape(D, 1)

    # host-exact rope tables per batch: row r (r%64 = freq) x position
    cosbt, sinbt = [], []
    for g in range(positions.shape[0]):
        ang = np.outer(invf, positions[g].astype(np.float64))  # [64, S]
        c = np.cos(ang).astype(BF)
        s = np.sin(ang).astype(BF)
        cosbt.append(np.concatenate([c, c], axis=0))
        sinbt.append(np.concatenate([s, s], axis=0))

    # hiddens st-major [ST*P, KO, 512] so each per-st DMA has 16KB
    # contiguous rows
    hT_st = []
    for g in range(hidden_states.shape[0]):
        a = hidden_states[g].T.reshape(KO, P, ST, 512)
        hT_st.append(
            np.ascontiguousarray(a.transpose(2, 1, 0, 3))
            .reshape(ST * P, KO, 512).astype(BF))

    in_maps = []
    for core in range(8):
        g, r = core // 4, core % 4
        wq_cols = w_qkv[:, 512 * r : 512 * (r + 1)]
        wk_col = w_qkv[:, 2048 + 128 * r : 2048 + 128 * (r + 1)]
        wv_col = w_qkv[:, 2560 + 128 * r : 2560 + 128 * (r + 1)]
        # weights ko-major [P, KO, cols]: 8-16KB contiguous DMA rows
        wkv_h = np.concatenate([wk_col, wv_col], axis=1)  # [H, 256]
        wkv_h = np.ascontiguousarray(
            wkv_h.reshape(KO, P, 2 * D).transpose(1, 0, 2)).astype(BF)
        wqz_h = np.ascontiguousarray(
            wq_cols.reshape(KO, P, QW).transpose(1, 0, 2)).astype(BF)
        in_maps.append(
            {
                "hT": hT_st[g],
                "wkv": wkv_h,
                "wqz": wqz_h,
                "wo": np.ascontiguousarray(
                    w_o[512 * r : 512 * (r + 1), :]
                ).astype(BF),
                "cosbt": cosbt[g],
                "sinbt": sinbt[g],
                "wqn": wqn,
                "wkn": wkn,
                "maskneg": maskneg,
                "identr": identr,
                "rmat": rmat,
            }
        )
    return in_maps


def run(trace=False, **inputs):
    nc = _get_nc()
    in_maps = _host_inputs(**inputs)
    res = run_bass_kernel_spmd(nc, in_maps, core_ids=list(range(8)), trace=trace)
    B = inputs["hidden_states"].shape[0]
    out = np.zeros((B, S, H), dtype=np.float64)
    for core in range(8):
        g = core // 4
        out[g] += res.results[core]["out"].astype(np.float64)
    return out.astype(np.float32), res


def kernel(**inputs):
    out, _ = run(trace=False, **inputs)
    return out
